# revision 5
# baseline (speedup 1.0000x reference)
"""CantorAttention Trainium2 kernel (8 NeuronCores, SPMD).

Strategy
--------
Shard (batch=2) x (head-pairs=4) across the 8 cores: core c handles batch
c//4 and heads {2*(c%4), 2*(c%4)+1}.  QKV projection is column-sharded,
output projection row-sharded per head pair; partial outputs are summed on
host.

The sparse gather `k[:, :, routes, :]` is turned into *dense band attention*
by a host-side permutation: sorting positions so that each query's K=64
routed keys fall in a small contiguous window (for the Cantor-route
structure, a 128-aligned window of <=3 x 128 keys per 128-query tile).
Duplicate / arbitrary routes are handled exactly via a per-(query,key)
count mask multiplied into exp(scores); the window degrades gracefully up
to the full dense 2048 keys for unstructured routes.

Device dataflow per core (all bf16 compute, f32 PSUM accumulate):
  xT (512,2048)  = x[b].T with permuted columns (host-prepped)
  qkT = Wqk.T @ xT          -> q^T,k^T with head_dim on partitions
  v   = xT.T @ Wv           -> natural (seq, 64*2) + ones column for Z
  per (query-tile t, head h):
    S^T chunk = k^T_band.T @ q^T_tile      (PE)
    P^T = exp(S * scale) * count_mask      (ACT exp + DVE mul)
    attn = P^T.T @ [V | 1] -> out + Z      (PE, accumulated over chunks)
    attn = attn * (1/Z)                    (DVE)
  aT = attn^T (PE transpose), out^T = Wout.T-chunks @ aT (PE)
  DMA out^T (512, 2048) bf16; host un-permutes, sums partials, adds biases.
"""

import numpy as np
import ml_dtypes

import concourse.bass as bass
import concourse.tile as tile
from concourse import bacc, mybir, masks
from concourse.bass_utils import run_bass_kernel_spmd

BF16 = ml_dtypes.bfloat16
B, S, DIM, H, HD, KNN = 2, 2048, 512, 8, 64, 64
NCORES = 8
T = 128           # queries per tile
NT = S // T       # 16 query tiles
SCALE = 1.0 / float(np.sqrt(HD))
CCH = DIM // 128  # 4 contraction chunks of the model dim


# ----------------------------------------------------------------------------
# Host-side planning: permutation + per-tile key windows + count masks
# ----------------------------------------------------------------------------

def _cantor_perm() -> np.ndarray:
    """Sort order of positions by their Cantor-set coordinate (the structure
    the reference's routes are built from)."""
    x = np.arange(S, dtype=np.float64) / max(1, S - 1)
    x = np.clip(x, 1e-06, 1.0 - 1e-06)
    val = np.zeros(S, dtype=np.float64)
    factor = 0.5
    for _ in range(8):
        x *= 3.0
        digit = np.floor(x)
        x -= digit
        val += (digit == 2.0) * factor
        factor *= 0.5
    return np.argsort(val.astype(np.float32), kind="stable")


def _windows_for(perm: np.ndarray, routes: np.ndarray):
    """Per query-tile 128-aligned key windows in permuted space."""
    inv = np.empty(S, np.int64)
    inv[perm] = np.arange(S)
    r_q = inv[routes][perm]  # (S, K): sorted-query -> sorted key positions
    lo = np.empty(NT, np.int64)
    nkc = np.empty(NT, np.int64)
    for t in range(NT):
        blk = r_q[t * T:(t + 1) * T]
        lo[t] = (blk.min() // T) * T
        nkc[t] = -(-(blk.max() + 1 - lo[t]) // T)
    return r_q, lo, nkc


def _plan(routes: np.ndarray):
    candidates = [
        _cantor_perm(),
        np.arange(S),
        np.argsort(routes.min(axis=1), kind="stable"),
        np.argsort(np.median(routes, axis=1), kind="stable"),
    ]
    best = None
    for perm in candidates:
        r_q, lo, nkc = _windows_for(perm, routes)
        cost = int(nkc.sum())
        if best is None or cost < best[0]:
            best = (cost, perm, r_q, lo, nkc)
    _, perm, r_q, lo, nkc = best

    # count masks, transposed: for tile t, chunk kc: mask[key_in_chunk, query]
    off = np.zeros(NT, np.int64)
    off[1:] = np.cumsum(nkc)[:-1]
    total = int(nkc.sum())
    maskT = np.zeros((total, T, T), np.float32)
    for t in range(NT):
        blk = r_q[t * T:(t + 1) * T]  # (T, K) key positions
        rel = blk - lo[t]
        chunk = rel // T              # which chunk each key falls in
        w = rel % T
        q_idx = np.broadcast_to(np.arange(T)[:, None], blk.shape)
        np.add.at(maskT, (off[t] + chunk, w, q_idx), 1.0)
    return perm, lo, nkc, off, maskT.astype(BF16)


# ----------------------------------------------------------------------------
# Device program
# ----------------------------------------------------------------------------

def _build(lo, nkc, off, total_chunks, with_qk_bias):
    f32 = mybir.dt.float32
    bf16 = mybir.dt.bfloat16
    nc = bacc.Bacc("TRN2", target_bir_lowering=False, debug=False,
                   num_devices=NCORES)

    xT_d = nc.dram_tensor("xT", [DIM, S], bf16, kind="ExternalInput").ap()
    wqk_d = nc.dram_tensor("wqk", [DIM, 256], bf16, kind="ExternalInput").ap()
    wv_d = nc.dram_tensor("wv", [DIM, 128], bf16, kind="ExternalInput").ap()
    wout_d = nc.dram_tensor("wout", [128, DIM], bf16, kind="ExternalInput").ap()
    maskT_d = nc.dram_tensor("maskT", [total_chunks, T, T], bf16,
                             kind="ExternalInput").ap()
    if with_qk_bias:
        bqk_d = nc.dram_tensor("bqk", [256, 1], f32, kind="ExternalInput").ap()
    outT_d = nc.dram_tensor("outT", [DIM, S], bf16, kind="ExternalOutput").ap()

    with tile.TileContext(nc) as tc:
        with (
            tc.tile_pool(name="persist", bufs=1) as persist,
            tc.tile_pool(name="ps_big", bufs=2, space="PSUM") as ps_big,
            tc.tile_pool(name="ps_s", bufs=3, space="PSUM") as ps_s,
            tc.tile_pool(name="ps_o", bufs=2, space="PSUM") as ps_o,
            tc.tile_pool(name="ps_t", bufs=1, space="PSUM") as ps_t,
            tc.tile_pool(name="mask", bufs=8) as maskp,
            tc.tile_pool(name="pt", bufs=3) as ptp,
            tc.tile_pool(name="small", bufs=4) as smallp,
        ):
            xT = persist.tile([128, CCH * S], bf16, tag="xT")
            qkT = persist.tile([128, 2 * S], bf16, tag="qkT")
            VSTR = 132
            v_sb = persist.tile([128, NT * VSTR], bf16, tag="v")
            wqk = persist.tile([128, CCH * 256], bf16, tag="wqk")
            wv = persist.tile([128, CCH * 128], bf16, tag="wv")
            wout = persist.tile([128, DIM], bf16, tag="wout")
            outT = persist.tile([128, CCH * S], bf16, tag="outT")
            ident = persist.tile([128, 128], bf16, tag="ident")

            masks.make_identity(nc, ident[:])

            # ---- input DMAs (c-chunked along partitions) ----
            nc.sync.dma_start(
                xT[:].rearrange("p (c f) -> p c f", c=CCH),
                xT_d.rearrange("(c p) f -> p c f", p=128))
            nc.sync.dma_start(
                wqk[:].rearrange("p (c f) -> p c f", c=CCH),
                wqk_d.rearrange("(c p) f -> p c f", p=128))
            nc.sync.dma_start(
                wv[:].rearrange("p (c f) -> p c f", c=CCH),
                wv_d.rearrange("(c p) f -> p c f", p=128))
            nc.sync.dma_start(wout[:], wout_d)
            if with_qk_bias:
                bqk = persist.tile([128, 2], f32, tag="bqk")
                nc.sync.dma_start(
                    bqk[:].rearrange("p (c f) -> p c f", c=2),
                    bqk_d.rearrange("(c p) f -> p c f", p=128))

            # ---- stage A: qkT = Wqk.T @ xT ----
            for f in range(2):
                for n in range(4):
                    ps = ps_big.tile([128, 512], f32, tag="A")
                    for c in range(CCH):
                        nc.tensor.matmul(
                            ps[:],
                            lhsT=wqk[:, c * 256 + f * 128:c * 256 + (f + 1) * 128],
                            rhs=xT[:, c * S + n * 512:c * S + (n + 1) * 512],
                            start=(c == 0), stop=(c == CCH - 1))
                    dst = qkT[:, f * S + n * 512:f * S + (n + 1) * 512]
                    if with_qk_bias:
                        nc.vector.tensor_scalar_add(dst, ps[:], bqk[:, f:f + 1])
                    elif f == 0:
                        nc.scalar.copy(dst, ps[:])
                    else:
                        nc.vector.tensor_copy(dst, ps[:])

            # ---- stage B: v = xT.T @ Wv ----
            # block layout: [v_h0 64 | ones 2 | v_h1 64 | ones 2] so that a
            # single N=65 matmul per head yields [out | Z] in one PSUM
            # accumulation group (interleaved groups in one bank are broken).
            nc.vector.memset(v_sb[:], 1.0)
            for g in range(NT):
                psv = ps_s.tile([128, 128], f32, tag="S")
                for c in range(CCH):
                    nc.tensor.matmul(
                        psv[:],
                        lhsT=xT[:, c * S + g * 128:c * S + (g + 1) * 128],
                        rhs=wv[:, c * 128:(c + 1) * 128],
                        start=(c == 0), stop=(c == CCH - 1))
                nc.vector.tensor_copy(
                    v_sb[:, g * VSTR:g * VSTR + 64], psv[:, 0:64])
                nc.vector.tensor_copy(
                    v_sb[:, g * VSTR + 66:g * VSTR + 130], psv[:, 64:128])

            # ---- stage C: band attention per (tile, head) ----
            for t in range(NT):
                nk = int(nkc[t])
                mts = []
                for kc in range(nk):
                    mt = maskp.tile([128, 128], bf16)
                    nc.sync.dma_start(mt[:], maskT_d[int(off[t]) + kc])
                    mts.append(mt)
                attn = smallp.tile([128, 128], bf16, tag="attn")
                for h in range(2):
                    hp = h * 64
                    pt = ptp.tile([128, nk * 128], bf16, tag="pt")
                    for kc in range(nk):
                        pss = ps_s.tile([128, 128], f32, tag="S")
                        nc.tensor.matmul(
                            pss[:],
                            lhsT=qkT[hp:hp + 64,
                                     S + int(lo[t]) + kc * 128:
                                     S + int(lo[t]) + (kc + 1) * 128],
                            rhs=qkT[hp:hp + 64, t * 128:(t + 1) * 128],
                            start=True, stop=True)
                        pslice = pt[:, kc * 128:(kc + 1) * 128]
                        nc.scalar.activation(
                            pslice, pss[:],
                            mybir.ActivationFunctionType.Exp, scale=SCALE)
                        nc.vector.tensor_mul(pslice, pslice, mts[kc][:])
                    pso = ps_o.tile([128, 68], f32, tag="O")
                    for kc in range(nk):
                        g = int(lo[t]) // 128 + kc
                        nc.tensor.matmul(
                            pso[:, 0:65],
                            lhsT=pt[:, kc * 128:(kc + 1) * 128],
                            rhs=v_sb[:, g * VSTR + h * 66:g * VSTR + h * 66 + 65],
                            start=(kc == 0), stop=(kc == nk - 1))
                    rz = smallp.tile([128, 1], f32, tag="rz")
                    nc.vector.reciprocal(rz[:], pso[:, 64:65])
                    nc.vector.tensor_scalar_mul(
                        attn[:, hp:hp + 64], pso[:, 0:64], rz[:])

                # ---- stage D: transpose + output projection ----
                pst = ps_t.tile([128, 128], bf16, tag="T")
                nc.tensor.transpose(pst[:], attn[:], ident[:])
                aT = smallp.tile([128, 128], bf16, tag="aT")
                nc.scalar.copy(aT[:], pst[:])
                for oc in range(CCH):
                    psp = ps_s.tile([128, 128], f32, tag="S")
                    nc.tensor.matmul(
                        psp[:],
                        lhsT=wout[:, oc * 128:(oc + 1) * 128],
                        rhs=aT[:], start=True, stop=True)
                    dst = outT[:, oc * S + t * 128:oc * S + (t + 1) * 128]
                    if oc % 2 == 0:
                        nc.scalar.copy(dst, psp[:])
                    else:
                        nc.vector.tensor_copy(dst, psp[:])

            nc.sync.dma_start(
                outT_d.rearrange("(c p) f -> p c f", p=128),
                outT[:].rearrange("p (c f) -> p c f", c=CCH))

    nc.compile()
    return nc


_CACHE = {}


def _get_program(lo, nkc, off, total_chunks, with_qk_bias):
    key = (tuple(int(v) for v in lo), tuple(int(v) for v in nkc),
           bool(with_qk_bias))
    if key not in _CACHE:
        _CACHE[key] = _build(lo, nkc, off, total_chunks, with_qk_bias)
    return _CACHE[key]


# ----------------------------------------------------------------------------
# Entry point
# ----------------------------------------------------------------------------

def kernel(x, Wqkv, bqkv, Wout, bout, routes):
    x = np.asarray(x, np.float32)
    Wqkv = np.asarray(Wqkv, np.float32)
    bqkv = np.asarray(bqkv, np.float32)
    Wout = np.asarray(Wout, np.float32)
    bout = np.asarray(bout, np.float32)
    routes = np.asarray(routes)

    perm, lo, nkc, off, maskT = _plan(routes)
    total_chunks = int(nkc.sum())

    bq = bqkv[0:DIM]
    bk = bqkv[DIM:2 * DIM]
    bv = bqkv[2 * DIM:3 * DIM]
    with_qk_bias = bool(np.any(bq) or np.any(bk))

    nc = _get_program(lo, nkc, off, total_chunks, with_qk_bias)

    maskT_flat = np.ascontiguousarray(maskT)
    in_maps = []
    for c in range(NCORES):
        b = c // 4
        h0 = 2 * (c % 4)
        cols_q = slice(h0 * HD, (h0 + 2) * HD)
        wqk = np.concatenate(
            [Wqkv[:, cols_q], Wqkv[:, DIM + h0 * HD:DIM + (h0 + 2) * HD]],
            axis=1)
        m = {
            "xT": np.ascontiguousarray(x[b].T[:, perm]).astype(BF16),
            "wqk": np.ascontiguousarray(wqk).astype(BF16),
            "wv": np.ascontiguousarray(
                Wqkv[:, 2 * DIM + h0 * HD:2 * DIM + (h0 + 2) * HD]).astype(BF16),
            "wout": np.ascontiguousarray(
                Wout[h0 * HD:(h0 + 2) * HD, :]).astype(BF16),
            "maskT": maskT_flat,
        }
        if with_qk_bias:
            m["bqk"] = np.concatenate(
                [bq[h0 * HD:(h0 + 2) * HD],
                 bk[h0 * HD:(h0 + 2) * HD]]).reshape(256, 1).astype(np.float32)
        in_maps.append(m)

    global _last_in_maps
    _last_in_maps = in_maps
    res = run_bass_kernel_spmd(nc, in_maps, core_ids=list(range(NCORES)))

    out = np.zeros((B, S, DIM), np.float32)
    for c in range(NCORES):
        b = c // 4
        part = res.results[c]["outT"].astype(np.float32).T  # (S, DIM) permuted
        out[b][perm] += part
    out += bout[None, None, :]
    if np.any(bv):
        out += (bv @ Wout)[None, None, :]
    return out


# revision 9
# speedup vs baseline: 1.3645x; 1.3645x over previous
"""CantorAttention Trainium2 kernel (8 NeuronCores, SPMD).

Strategy
--------
Shard (batch=2) x (head-pairs=4) across the 8 cores: core c handles batch
c//4 and heads {2*(c%4), 2*(c%4)+1}.  QKV projection is column-sharded,
output projection row-sharded per head pair; partial outputs are summed on
host.

The sparse gather `k[:, :, routes, :]` is turned into *dense band attention*
by a host-side permutation: sorting positions so that each query's K=64
routed keys fall in a small contiguous window (for the Cantor-route
structure, a 128-aligned window of <=3 x 128 keys per 128-query tile).
Duplicate / arbitrary routes are handled exactly via a per-(query,key)
count mask multiplied into exp(scores); unstructured routes degrade
gracefully to the full dense 2048-key window.

Device dataflow per core (bf16 compute, f32 PSUM accumulate):
  xT (512,2048)  = x[b].T with permuted columns (host-prepped)
  qkvT = Wqkv_c.T @ xT      -> q^T,k^T,v^T with head_dim on partitions
  v    = per-128 transpose of v^T (+ ones columns for the softmax Z)
  key-chunk-major scores: for key chunk g, the covering queries form a
  contiguous range (<=512 wide):  S^T = k^T_g.T @ q^T_range   (one matmul)
    P^T = exp(S*scale) * count_mask                           (ACT + DVE)
  per query tile t (once its last chunk is done), per head:
    attn_unnorm | Z = P^T_chunks.T @ [V | 1]   (PE, accumulated)
    attn = attn_unnorm * (1/Z)                 (DVE)
  groups of 4 tiles: aT = attn^T (PE transpose), out^T = Wout-chunks @ aT
  DMA out^T (512, 2048) bf16; host un-permutes, sums partials, adds biases.
"""

import numpy as np
import ml_dtypes

import concourse.bass as bass
import concourse.tile as tile
from concourse import bacc, mybir, masks
from concourse.bass_utils import run_bass_kernel_spmd

BF16 = ml_dtypes.bfloat16
B, S, DIM, H, HD, KNN = 2, 2048, 512, 8, 64, 64
NCORES = 8
T = 128           # queries per tile
NT = S // T       # 16 query tiles
NG = S // T       # 16 key chunks
SCALE = 1.0 / float(np.sqrt(HD))
CCH = DIM // 128  # 4 contraction chunks of the model dim
VSTR = 132        # v block stride: [v_h0 64 | ones 2 | v_h1 64 | ones 2]


# ----------------------------------------------------------------------------
# Host-side planning: permutation + per-tile key windows + count masks
# ----------------------------------------------------------------------------

def _cantor_perm() -> np.ndarray:
    """Sort order of positions by their Cantor-set coordinate (the structure
    the reference's routes are built from)."""
    x = np.arange(S, dtype=np.float64) / max(1, S - 1)
    x = np.clip(x, 1e-06, 1.0 - 1e-06)
    val = np.zeros(S, dtype=np.float64)
    factor = 0.5
    for _ in range(8):
        x *= 3.0
        digit = np.floor(x)
        x -= digit
        val += (digit == 2.0) * factor
        factor *= 0.5
    return np.argsort(val.astype(np.float32), kind="stable")


def _windows_for(perm: np.ndarray, routes: np.ndarray):
    inv = np.empty(S, np.int64)
    inv[perm] = np.arange(S)
    r_q = inv[routes][perm]  # (S, K): sorted-query -> sorted key positions
    lo = np.empty(NT, np.int64)
    nkc = np.empty(NT, np.int64)
    for t in range(NT):
        blk = r_q[t * T:(t + 1) * T]
        lo[t] = (blk.min() // T) * T
        nkc[t] = -(-(blk.max() + 1 - lo[t]) // T)
    return r_q, lo, nkc


class Plan:
    pass


def _plan(routes: np.ndarray) -> Plan:
    candidates = [
        _cantor_perm(),
        np.arange(S),
        np.argsort(routes.min(axis=1), kind="stable"),
        np.argsort(np.median(routes, axis=1), kind="stable"),
    ]
    best = None
    for perm in candidates:
        r_q, lo, nkc = _windows_for(perm, routes)
        cost = int(nkc.sum())
        if best is None or cost < best[0]:
            best = (cost, perm, r_q, lo, nkc)
    _, perm, r_q, lo, nkc = best

    p = Plan()
    p.perm, p.lo, p.nkc = perm, lo, nkc

    # tiles covering each key chunk g
    cover = [[] for _ in range(NG)]
    for t in range(NT):
        for kc in range(int(nkc[t])):
            cover[int(lo[t]) // T + kc].append(t)

    # score jobs: (g, t0, nt) pieces with nt <= 4 (N <= 512)
    jobs = []            # list of (g, t0, nt, block_base)
    piece_of = {}        # (g, t) -> (job_idx, t0)
    nblocks = 0
    for g in range(NG):
        ts = cover[g]
        if not ts:
            continue
        assert ts == list(range(ts[0], ts[0] + len(ts))), \
            f"non-contiguous cover for chunk {g}: {ts}"
        i = 0
        while i < len(ts):
            nt = min(4, len(ts) - i)
            t0 = ts[i]
            jidx = len(jobs)
            jobs.append((g, t0, nt, nblocks))
            for t in ts[i:i + nt]:
                piece_of[(g, t)] = (jidx, t0)
            nblocks += nt
            i += nt
    p.jobs, p.piece_of, p.nblocks = jobs, piece_of, nblocks

    # PV emission: tile t is ready after its last chunk's job
    last_g = {t: int(lo[t]) // T + int(nkc[t]) - 1 for t in range(NT)}
    p.emit_after_job = [[] for _ in range(len(jobs))]
    for t in range(NT):
        jidx = piece_of[(last_g[t], t)][0]
        # also require all earlier jobs of t done; jobs are emitted in order
        jmax = max(piece_of[(int(lo[t]) // T + kc, t)][0]
                   for kc in range(int(nkc[t])))
        p.emit_after_job[jmax].append(t)

    # peak live score-piece tiles (per head) for pool sizing:
    # window of job indices alive simultaneously
    alive_until = {}
    for jidx, (g, t0, nt, _) in enumerate(jobs):
        last = jidx
        for t in range(t0, t0 + nt):
            last = max(last, max(piece_of[(int(lo[t]) // T + kc, t)][0]
                                 for kc in range(int(nkc[t]))))
        alive_until[jidx] = last
    peak = 0
    for j in range(len(jobs)):
        peak = max(peak, sum(1 for jj, lu in alive_until.items()
                             if jj <= j <= lu))
    p.peak_live = peak

    # count masks, g-major blocks: block b (for job piece, tile t) is
    # mask[key_in_chunk, query_in_tile]
    maskG = np.zeros((nblocks, T, T), np.float32)
    inv = np.empty(S, np.int64)
    inv[perm] = np.arange(S)
    for g, t0, nt, base in jobs:
        for j, t in enumerate(range(t0, t0 + nt)):
            blk = r_q[t * T:(t + 1) * T]
            sel = (blk // T) == g
            w = (blk % T)[sel]
            q_idx = np.broadcast_to(np.arange(T)[:, None], blk.shape)[sel]
            np.add.at(maskG, (base + j, w, q_idx), 1.0)
    p.maskG = maskG.astype(BF16)
    return p


# ----------------------------------------------------------------------------
# Device program
# ----------------------------------------------------------------------------

def _build(p: Plan, with_qk_bias: bool):
    f32 = mybir.dt.float32
    bf16 = mybir.dt.bfloat16
    lo, nkc = p.lo, p.nkc
    nc = bacc.Bacc("TRN2", target_bir_lowering=False, debug=False,
                   num_devices=NCORES)

    xT_d = nc.dram_tensor("xT", [DIM, S], bf16, kind="ExternalInput").ap()
    wqkv_d = nc.dram_tensor("wqkv", [DIM, 384], bf16, kind="ExternalInput").ap()
    wout_d = nc.dram_tensor("wout", [128, DIM], bf16, kind="ExternalInput").ap()
    maskG_d = nc.dram_tensor("maskG", [p.nblocks, T, T], bf16,
                             kind="ExternalInput").ap()
    if with_qk_bias:
        bqk_d = nc.dram_tensor("bqk", [256, 1], f32, kind="ExternalInput").ap()
    outT_d = nc.dram_tensor("outT", [DIM, S], bf16, kind="ExternalOutput").ap()

    ptg_bufs = max(6, min(2 * p.peak_live + 2, 16))

    with tile.TileContext(nc) as tc:
        with (
            tc.tile_pool(name="persist", bufs=1) as persist,
            tc.tile_pool(name="ps_a", bufs=4, space="PSUM") as ps_a,
            tc.tile_pool(name="ps_o", bufs=3, space="PSUM") as ps_o,
            tc.tile_pool(name="ps_t", bufs=1, space="PSUM") as ps_t,
            tc.tile_pool(name="mask", bufs=6) as maskp,
            tc.tile_pool(name="ptg", bufs=ptg_bufs) as ptgp,
            tc.tile_pool(name="small", bufs=4) as smallp,
        ):
            xT = persist.tile([128, CCH * S], bf16, tag="xT")
            qkT = persist.tile([128, 2 * S], bf16, tag="qkT")
            vT = persist.tile([128, S], bf16, tag="vT")
            v_sb = persist.tile([128, NT * VSTR], bf16, tag="v")
            wqkv = persist.tile([128, CCH * 384], bf16, tag="wqkv")
            wout = persist.tile([128, DIM], bf16, tag="wout")
            outT = persist.tile([128, CCH * S], bf16, tag="outT")
            ident = persist.tile([128, 128], bf16, tag="ident")

            masks.make_identity(nc, ident[:])

            nc.sync.dma_start(
                xT[:].rearrange("p (c f) -> p c f", c=CCH),
                xT_d.rearrange("(c p) f -> p c f", p=128))
            nc.sync.dma_start(
                wqkv[:].rearrange("p (c f) -> p c f", c=CCH),
                wqkv_d.rearrange("(c p) f -> p c f", p=128))
            nc.sync.dma_start(wout[:], wout_d)
            if with_qk_bias:
                bqk = persist.tile([128, 2], f32, tag="bqk")
                nc.sync.dma_start(
                    bqk[:].rearrange("p (c f) -> p c f", c=2),
                    bqk_d.rearrange("(c p) f -> p c f", p=128))

            # ---- stage A: qkvT = Wqkv_c.T @ xT  (3 f-tiles: q|k|v pairs) ----
            for f in range(3):
                for n in range(4):
                    ps = ps_a.tile([128, 512], f32, tag="A")
                    for c in range(CCH):
                        nc.tensor.matmul(
                            ps[:],
                            lhsT=wqkv[:, c * 384 + f * 128:c * 384 + (f + 1) * 128],
                            rhs=xT[:, c * S + n * 512:c * S + (n + 1) * 512],
                            start=(c == 0), stop=(c == CCH - 1))
                    if f < 2:
                        dst = qkT[:, f * S + n * 512:f * S + (n + 1) * 512]
                    else:
                        dst = vT[:, n * 512:(n + 1) * 512]
                    if with_qk_bias and f < 2:
                        nc.vector.tensor_scalar_add(dst, ps[:], bqk[:, f:f + 1])
                    elif (f * 4 + n) % 2 == 0:
                        nc.scalar.copy(dst, ps[:])
                    else:
                        nc.vector.tensor_copy(dst, ps[:])

            # ---- stage B: v natural blocks via PE transpose of vT ----
            nc.vector.memset(v_sb[:], 1.0)
            for g in range(NT):
                psv = ps_t.tile([128, 128], bf16, tag="T")
                nc.tensor.transpose(psv[:], vT[:, g * 128:(g + 1) * 128], ident[:])
                nc.scalar.copy(v_sb[:, g * VSTR:g * VSTR + 64], psv[:, 0:64])
                nc.scalar.copy(v_sb[:, g * VSTR + 66:g * VSTR + 130],
                               psv[:, 64:128])

            # ---- stage C: key-chunk-major scores + per-tile PV ----
            pt_tiles = {}      # (jidx, h) -> sbuf tile
            attn_tiles = {}    # t -> attn tile
            aT_wide = None
            for jidx, (g, t0, nt, base) in enumerate(p.jobs):
                nq = nt * 128
                mt = maskp.tile([128, 512], bf16, tag="mask")
                nc.sync.dma_start(
                    mt[:, 0:nq].rearrange("p (a f) -> p a f", a=nt),
                    maskG_d[base:base + nt].rearrange("a p f -> p a f"))
                for h in range(2):
                    hp = h * 64
                    pss = ps_a.tile([128, 512], f32, tag="A")
                    nc.tensor.matmul(
                        pss[:, 0:nq],
                        lhsT=qkT[hp:hp + 64, S + g * 128:S + (g + 1) * 128],
                        rhs=qkT[hp:hp + 64, t0 * 128:t0 * 128 + nq],
                        start=True, stop=True)
                    pt = ptgp.tile([128, 512], bf16, tag="ptg")
                    nc.scalar.activation(
                        pt[:, 0:nq], pss[:, 0:nq],
                        mybir.ActivationFunctionType.Exp, scale=SCALE)
                    nc.vector.tensor_mul(pt[:, 0:nq], pt[:, 0:nq], mt[:, 0:nq])
                    pt_tiles[(jidx, h)] = pt

                for t in p.emit_after_job[jidx]:
                    attn = smallp.tile([128, 128], bf16, tag="attn")
                    attn_tiles[t] = attn
                    for h in range(2):
                        pso = ps_o.tile([128, 68], f32, tag="O")
                        nk = int(nkc[t])
                        for kc in range(nk):
                            gg = int(lo[t]) // T + kc
                            jj, tt0 = p.piece_of[(gg, t)]
                            src = pt_tiles[(jj, h)]
                            coff = (t - tt0) * 128
                            nc.tensor.matmul(
                                pso[:, 0:65],
                                lhsT=src[:, coff:coff + 128],
                                rhs=v_sb[:, gg * VSTR + h * 66:
                                         gg * VSTR + h * 66 + 65],
                                start=(kc == 0), stop=(kc == nk - 1))
                        rz = smallp.tile([128, 1], f32, tag="rz")
                        nc.vector.reciprocal(rz[:], pso[:, 64:65])
                        nc.vector.tensor_scalar_mul(
                            attn[:, h * 64:h * 64 + 64], pso[:, 0:64], rz[:])

                    # transpose into the 4-tile-wide aT buffer
                    if t % 4 == 0:
                        aT_wide = smallp.tile([128, 512], bf16, tag="aTw")
                    pst = ps_t.tile([128, 128], bf16, tag="T")
                    nc.tensor.transpose(pst[:], attn[:], ident[:])
                    if t % 2 == 0:
                        nc.scalar.copy(aT_wide[:, (t % 4) * 128:(t % 4 + 1) * 128],
                                       pst[:])
                    else:
                        nc.vector.tensor_copy(
                            aT_wide[:, (t % 4) * 128:(t % 4 + 1) * 128], pst[:])

                    # ---- stage D: batched output projection ----
                    if t % 4 == 3:
                        tg = t // 4
                        for oc in range(CCH):
                            psp = ps_a.tile([128, 512], f32, tag="A")
                            nc.tensor.matmul(
                                psp[:],
                                lhsT=wout[:, oc * 128:(oc + 1) * 128],
                                rhs=aT_wide[:], start=True, stop=True)
                            dst = outT[:, oc * S + tg * 512:oc * S + (tg + 1) * 512]
                            if oc % 2 == 0:
                                nc.vector.tensor_copy(dst, psp[:])
                            else:
                                nc.scalar.copy(dst, psp[:])

            nc.sync.dma_start(
                outT_d.rearrange("(c p) f -> p c f", p=128),
                outT[:].rearrange("p (c f) -> p c f", c=CCH))

    nc.compile()
    return nc


_CACHE = {}


def _get_program(p: Plan, with_qk_bias: bool):
    key = (tuple(int(v) for v in p.lo), tuple(int(v) for v in p.nkc),
           bool(with_qk_bias))
    if key not in _CACHE:
        _CACHE[key] = _build(p, with_qk_bias)
    return _CACHE[key]


# ----------------------------------------------------------------------------
# Entry point
# ----------------------------------------------------------------------------

def kernel(x, Wqkv, bqkv, Wout, bout, routes):
    x = np.asarray(x, np.float32)
    Wqkv = np.asarray(Wqkv, np.float32)
    bqkv = np.asarray(bqkv, np.float32)
    Wout = np.asarray(Wout, np.float32)
    bout = np.asarray(bout, np.float32)
    routes = np.asarray(routes)

    p = _plan(routes)
    perm = p.perm

    bq = bqkv[0:DIM]
    bk = bqkv[DIM:2 * DIM]
    bv = bqkv[2 * DIM:3 * DIM]
    with_qk_bias = bool(np.any(bq) or np.any(bk))

    nc = _get_program(p, with_qk_bias)

    maskG_flat = np.ascontiguousarray(p.maskG)
    in_maps = []
    for c in range(NCORES):
        b = c // 4
        h0 = 2 * (c % 4)
        cols = slice(h0 * HD, (h0 + 2) * HD)
        wqkv = np.concatenate(
            [Wqkv[:, cols],
             Wqkv[:, DIM + h0 * HD:DIM + (h0 + 2) * HD],
             Wqkv[:, 2 * DIM + h0 * HD:2 * DIM + (h0 + 2) * HD]], axis=1)
        m = {
            "xT": np.ascontiguousarray(x[b].T[:, perm]).astype(BF16),
            "wqkv": np.ascontiguousarray(wqkv).astype(BF16),
            "wout": np.ascontiguousarray(
                Wout[h0 * HD:(h0 + 2) * HD, :]).astype(BF16),
            "maskG": maskG_flat,
        }
        if with_qk_bias:
            m["bqk"] = np.concatenate(
                [bq[h0 * HD:(h0 + 2) * HD],
                 bk[h0 * HD:(h0 + 2) * HD]]).reshape(256, 1).astype(np.float32)
        in_maps.append(m)

    global _last_in_maps
    _last_in_maps = in_maps
    res = run_bass_kernel_spmd(nc, in_maps, core_ids=list(range(NCORES)))

    out = np.zeros((B, S, DIM), np.float32)
    for c in range(NCORES):
        b = c // 4
        part = res.results[c]["outT"].astype(np.float32).T  # (S, DIM) permuted
        out[b][perm] += part
    out += bout[None, None, :]
    if np.any(bv):
        out += (bv @ Wout)[None, None, :]
    return out


# revision 11
# speedup vs baseline: 1.6971x; 1.2438x over previous
"""CantorAttention Trainium2 kernel (8 NeuronCores, SPMD).

Strategy
--------
Shard (batch=2) x (head-pairs=4) across the 8 cores: core c handles batch
c//4 and heads {2*(c%4), 2*(c%4)+1}.  QKV projection is column-sharded,
output projection row-sharded per head pair; partial outputs are summed on
host.

The sparse gather `k[:, :, routes, :]` is turned into *dense band attention*
by a host-side permutation: sorting positions so that each query's K=64
routed keys fall in a small contiguous window (for the Cantor-route
structure, a 128-aligned window of <=3 x 128 keys per 128-query tile).
Duplicate / arbitrary routes are handled exactly via a per-(query,key)
count mask multiplied into exp(scores); unstructured routes degrade
gracefully to the full dense 2048-key window.

Device dataflow per core (bf16 compute, f32 PSUM accumulate):
  xT (512,2048)  = x[b].T with permuted columns (host-prepped)
  qkvT = Wqkv_c.T @ xT      -> q^T,k^T,v^T with head_dim on partitions
  v    = per-128 transpose of v^T (+ ones columns for the softmax Z)
  key-chunk-major scores: for key chunk g, the covering queries form a
  contiguous range (<=512 wide):  S^T = k^T_g.T @ q^T_range   (one matmul)
    P^T = exp(S*scale) * count_mask                           (ACT + DVE)
  per query tile t (once its last chunk is done), per head:
    attn_unnorm | Z = P^T_chunks.T @ [V | 1]   (PE, accumulated)
    attn = attn_unnorm * (1/Z)                 (DVE)
  groups of 4 tiles: aT = attn^T (PE transpose), out^T = Wout-chunks @ aT
  DMA out^T (512, 2048) bf16; host un-permutes, sums partials, adds biases.
"""

import numpy as np
import ml_dtypes

import concourse.bass as bass
import concourse.tile as tile
from concourse import bacc, mybir, masks
from concourse.bass_utils import run_bass_kernel_spmd

BF16 = ml_dtypes.bfloat16
B, S, DIM, H, HD, KNN = 2, 2048, 512, 8, 64, 64
NCORES = 8
T = 128           # queries per tile
NT = S // T       # 16 query tiles
NG = S // T       # 16 key chunks
SCALE = 1.0 / float(np.sqrt(HD))
CCH = DIM // 128  # 4 contraction chunks of the model dim
VSTR = 132        # v block stride: [v_h0 64 | ones 2 | v_h1 64 | ones 2]


# ----------------------------------------------------------------------------
# Host-side planning: permutation + per-tile key windows + count masks
# ----------------------------------------------------------------------------

def _cantor_perm() -> np.ndarray:
    """Sort order of positions by their Cantor-set coordinate (the structure
    the reference's routes are built from)."""
    x = np.arange(S, dtype=np.float64) / max(1, S - 1)
    x = np.clip(x, 1e-06, 1.0 - 1e-06)
    val = np.zeros(S, dtype=np.float64)
    factor = 0.5
    for _ in range(8):
        x *= 3.0
        digit = np.floor(x)
        x -= digit
        val += (digit == 2.0) * factor
        factor *= 0.5
    return np.argsort(val.astype(np.float32), kind="stable")


def _windows_for(perm: np.ndarray, routes: np.ndarray):
    inv = np.empty(S, np.int64)
    inv[perm] = np.arange(S)
    r_q = inv[routes][perm]  # (S, K): sorted-query -> sorted key positions
    lo = np.empty(NT, np.int64)
    nkc = np.empty(NT, np.int64)
    for t in range(NT):
        blk = r_q[t * T:(t + 1) * T]
        lo[t] = (blk.min() // T) * T
        nkc[t] = -(-(blk.max() + 1 - lo[t]) // T)
    return r_q, lo, nkc


class Plan:
    pass


def _plan(routes: np.ndarray) -> Plan:
    candidates = [
        _cantor_perm(),
        np.arange(S),
        np.argsort(routes.min(axis=1), kind="stable"),
        np.argsort(np.median(routes, axis=1), kind="stable"),
    ]
    best = None
    for perm in candidates:
        r_q, lo, nkc = _windows_for(perm, routes)
        cost = int(nkc.sum())
        if best is None or cost < best[0]:
            best = (cost, perm, r_q, lo, nkc)
    _, perm, r_q, lo, nkc = best

    p = Plan()
    p.perm, p.lo, p.nkc = perm, lo, nkc

    # tiles covering each key chunk g
    cover = [[] for _ in range(NG)]
    for t in range(NT):
        for kc in range(int(nkc[t])):
            cover[int(lo[t]) // T + kc].append(t)

    # score jobs: (g, t0, nt) pieces with nt <= 4 (N <= 512)
    jobs = []            # list of (g, t0, nt, block_base)
    piece_of = {}        # (g, t) -> (job_idx, t0)
    nblocks = 0
    for g in range(NG):
        ts = cover[g]
        if not ts:
            continue
        assert ts == list(range(ts[0], ts[0] + len(ts))), \
            f"non-contiguous cover for chunk {g}: {ts}"
        i = 0
        while i < len(ts):
            nt = min(4, len(ts) - i)
            t0 = ts[i]
            jidx = len(jobs)
            jobs.append((g, t0, nt, nblocks))
            for t in ts[i:i + nt]:
                piece_of[(g, t)] = (jidx, t0)
            nblocks += nt
            i += nt
    p.jobs, p.piece_of, p.nblocks = jobs, piece_of, nblocks

    # PV emission: tile t is ready after its last chunk's job
    last_g = {t: int(lo[t]) // T + int(nkc[t]) - 1 for t in range(NT)}
    p.emit_after_job = [[] for _ in range(len(jobs))]
    for t in range(NT):
        jidx = piece_of[(last_g[t], t)][0]
        # also require all earlier jobs of t done; jobs are emitted in order
        jmax = max(piece_of[(int(lo[t]) // T + kc, t)][0]
                   for kc in range(int(nkc[t])))
        p.emit_after_job[jmax].append(t)

    # peak live score-piece tiles (per head) for pool sizing:
    # window of job indices alive simultaneously
    alive_until = {}
    for jidx, (g, t0, nt, _) in enumerate(jobs):
        last = jidx
        for t in range(t0, t0 + nt):
            last = max(last, max(piece_of[(int(lo[t]) // T + kc, t)][0]
                                 for kc in range(int(nkc[t]))))
        alive_until[jidx] = last
    peak = 0
    for j in range(len(jobs)):
        peak = max(peak, sum(1 for jj, lu in alive_until.items()
                             if jj <= j <= lu))
    p.peak_live = peak

    # count masks, g-major blocks: block b (for job piece, tile t) is
    # mask[key_in_chunk, query_in_tile]
    maskG = np.zeros((nblocks, T, T), np.float32)
    inv = np.empty(S, np.int64)
    inv[perm] = np.arange(S)
    for g, t0, nt, base in jobs:
        for j, t in enumerate(range(t0, t0 + nt)):
            blk = r_q[t * T:(t + 1) * T]
            sel = (blk // T) == g
            w = (blk % T)[sel]
            q_idx = np.broadcast_to(np.arange(T)[:, None], blk.shape)[sel]
            np.add.at(maskG, (base + j, w, q_idx), 1.0)
    p.maskG = maskG.astype(BF16)
    return p


# ----------------------------------------------------------------------------
# Device program
# ----------------------------------------------------------------------------

def _build(p: Plan, with_qk_bias: bool):
    f32 = mybir.dt.float32
    bf16 = mybir.dt.bfloat16
    lo, nkc = p.lo, p.nkc
    nc = bacc.Bacc("TRN2", target_bir_lowering=False, debug=False,
                   num_devices=NCORES)

    xT_d = nc.dram_tensor("xT", [DIM, S], bf16, kind="ExternalInput").ap()
    wqkv_d = nc.dram_tensor("wqkv", [DIM, 384], bf16, kind="ExternalInput").ap()
    wout_d = nc.dram_tensor("wout", [128, DIM], bf16, kind="ExternalInput").ap()
    maskG_d = nc.dram_tensor("maskG", [p.nblocks, T, T], bf16,
                             kind="ExternalInput").ap()
    if with_qk_bias:
        bqk_d = nc.dram_tensor("bqk", [256, 1], f32, kind="ExternalInput").ap()
    outT_d = nc.dram_tensor("outT", [DIM, S], bf16, kind="ExternalOutput").ap()

    ptg_bufs = max(6, min(2 * p.peak_live + 2, 16))

    with tile.TileContext(nc) as tc:
        with (
            tc.tile_pool(name="persist", bufs=1) as persist,
            tc.tile_pool(name="ps_a", bufs=4, space="PSUM") as ps_a,
            tc.tile_pool(name="ps_o", bufs=2, space="PSUM") as ps_o,
            tc.tile_pool(name="ps_t", bufs=2, space="PSUM") as ps_t,
            tc.tile_pool(name="mask", bufs=6) as maskp,
            tc.tile_pool(name="ptg", bufs=ptg_bufs) as ptgp,
            tc.tile_pool(name="small", bufs=4) as smallp,
        ):
            xT = persist.tile([128, CCH * S], bf16, tag="xT")
            qkT = persist.tile([128, 2 * S], bf16, tag="qkT")
            vT = persist.tile([128, S], bf16, tag="vT")
            v_sb = persist.tile([128, NT * VSTR], bf16, tag="v")
            wqkv = persist.tile([128, CCH * 384], bf16, tag="wqkv")
            wout = persist.tile([128, DIM], bf16, tag="wout")
            outT = persist.tile([128, CCH * S], bf16, tag="outT")
            ident = persist.tile([128, 128], bf16, tag="ident")

            masks.make_identity(nc, ident[:])

            nc.sync.dma_start(
                wqkv[:].rearrange("p (c f) -> p c f", c=CCH),
                wqkv_d.rearrange("(c p) f -> p c f", p=128))
            for c in range(CCH):
                nc.sync.dma_start(xT[:, c * S:(c + 1) * S],
                                  xT_d[c * 128:(c + 1) * 128, :])
            nc.sync.dma_start(wout[:], wout_d)
            if with_qk_bias:
                bqk = persist.tile([128, 2], f32, tag="bqk")
                nc.sync.dma_start(
                    bqk[:].rearrange("p (c f) -> p c f", c=2),
                    bqk_d.rearrange("(c p) f -> p c f", p=128))

            # ---- stage A: qkvT = Wqkv_c.T @ xT  (3 f-tiles: q|k|v pairs) ----
            for f in (2, 0, 1):
                for n in range(4):
                    ps = ps_a.tile([128, 512], f32, tag="A")
                    for c in range(CCH):
                        nc.tensor.matmul(
                            ps[:],
                            lhsT=wqkv[:, c * 384 + f * 128:c * 384 + (f + 1) * 128],
                            rhs=xT[:, c * S + n * 512:c * S + (n + 1) * 512],
                            start=(c == 0), stop=(c == CCH - 1))
                    if f < 2:
                        dst = qkT[:, f * S + n * 512:f * S + (n + 1) * 512]
                    else:
                        dst = vT[:, n * 512:(n + 1) * 512]
                    if with_qk_bias and f < 2:
                        nc.vector.tensor_scalar_add(dst, ps[:], bqk[:, f:f + 1])
                    else:
                        nc.scalar.copy(dst, ps[:])

            # ---- stage B: v natural blocks via PE transpose of vT ----
            nc.gpsimd.memset(v_sb[:], 1.0)
            for g in range(NT):
                psv = ps_t.tile([128, 128], bf16, tag="T")
                nc.tensor.transpose(psv[:], vT[:, g * 128:(g + 1) * 128], ident[:])
                nc.vector.tensor_copy(v_sb[:, g * VSTR:g * VSTR + 64],
                                      psv[:, 0:64])
                nc.vector.tensor_copy(v_sb[:, g * VSTR + 66:g * VSTR + 130],
                                      psv[:, 64:128])

            # ---- stage C: key-chunk-major scores + per-tile PV ----
            pt_tiles = {}      # (jidx, h) -> sbuf tile
            attn_tiles = {}    # t -> attn tile
            aT_wide = None
            for jidx, (g, t0, nt, base) in enumerate(p.jobs):
                nq = nt * 128
                mt = maskp.tile([128, 512], bf16, tag="mask")
                nc.sync.dma_start(
                    mt[:, 0:nq].rearrange("p (a f) -> p a f", a=nt),
                    maskG_d[base:base + nt].rearrange("a p f -> p a f"))
                for h in range(2):
                    hp = h * 64
                    pss = ps_a.tile([128, 512], f32, tag="A")
                    nc.tensor.matmul(
                        pss[:, 0:nq],
                        lhsT=qkT[hp:hp + 64, S + g * 128:S + (g + 1) * 128],
                        rhs=qkT[hp:hp + 64, t0 * 128:t0 * 128 + nq],
                        start=True, stop=True)
                    pt = ptgp.tile([128, 512], bf16, tag="ptg")
                    nc.scalar.activation(
                        pt[:, 0:nq], pss[:, 0:nq],
                        mybir.ActivationFunctionType.Exp, scale=SCALE)
                    nc.vector.tensor_mul(pt[:, 0:nq], pt[:, 0:nq], mt[:, 0:nq])
                    pt_tiles[(jidx, h)] = pt

                for t in p.emit_after_job[jidx]:
                    attn = smallp.tile([128, 128], bf16, tag="attn")
                    attn_tiles[t] = attn
                    for h in range(2):
                        pso = ps_o.tile([128, 68], f32, tag="O")
                        nk = int(nkc[t])
                        for kc in range(nk):
                            gg = int(lo[t]) // T + kc
                            jj, tt0 = p.piece_of[(gg, t)]
                            src = pt_tiles[(jj, h)]
                            coff = (t - tt0) * 128
                            nc.tensor.matmul(
                                pso[:, 0:65],
                                lhsT=src[:, coff:coff + 128],
                                rhs=v_sb[:, gg * VSTR + h * 66:
                                         gg * VSTR + h * 66 + 65],
                                start=(kc == 0), stop=(kc == nk - 1))
                        rz = smallp.tile([128, 1], f32, tag="rz")
                        nc.vector.reciprocal(rz[:], pso[:, 64:65])
                        nc.vector.tensor_scalar_mul(
                            attn[:, h * 64:h * 64 + 64], pso[:, 0:64], rz[:])

                    # transpose into the 4-tile-wide aT buffer
                    if t % 4 == 0:
                        aT_wide = smallp.tile([128, 512], bf16, tag="aTw")
                    pst = ps_t.tile([128, 128], bf16, tag="T")
                    nc.tensor.transpose(pst[:], attn[:], ident[:])
                    nc.scalar.copy(
                        aT_wide[:, (t % 4) * 128:(t % 4 + 1) * 128], pst[:])

                    # ---- stage D: batched output projection ----
                    if t % 4 == 3:
                        tg = t // 4
                        for oc in range(CCH):
                            psp = ps_a.tile([128, 512], f32, tag="A")
                            nc.tensor.matmul(
                                psp[:],
                                lhsT=wout[:, oc * 128:(oc + 1) * 128],
                                rhs=aT_wide[:], start=True, stop=True)
                            dst = outT[:, oc * S + tg * 512:oc * S + (tg + 1) * 512]
                            nc.vector.tensor_copy(dst, psp[:])
                        nc.sync.dma_start(
                            outT_d.rearrange("(c p) f -> p c f", p=128)
                                  [:, :, tg * 512:(tg + 1) * 512],
                            outT[:].rearrange("p (c f) -> p c f", c=CCH)
                                [:, :, tg * 512:(tg + 1) * 512])

    nc.compile()
    return nc


_CACHE = {}


def _get_program(p: Plan, with_qk_bias: bool):
    key = (tuple(int(v) for v in p.lo), tuple(int(v) for v in p.nkc),
           bool(with_qk_bias))
    if key not in _CACHE:
        _CACHE[key] = _build(p, with_qk_bias)
    return _CACHE[key]


# ----------------------------------------------------------------------------
# Entry point
# ----------------------------------------------------------------------------

def kernel(x, Wqkv, bqkv, Wout, bout, routes):
    x = np.asarray(x, np.float32)
    Wqkv = np.asarray(Wqkv, np.float32)
    bqkv = np.asarray(bqkv, np.float32)
    Wout = np.asarray(Wout, np.float32)
    bout = np.asarray(bout, np.float32)
    routes = np.asarray(routes)

    p = _plan(routes)
    perm = p.perm

    bq = bqkv[0:DIM]
    bk = bqkv[DIM:2 * DIM]
    bv = bqkv[2 * DIM:3 * DIM]
    with_qk_bias = bool(np.any(bq) or np.any(bk))

    nc = _get_program(p, with_qk_bias)

    maskG_flat = np.ascontiguousarray(p.maskG)
    in_maps = []
    for c in range(NCORES):
        b = c // 4
        h0 = 2 * (c % 4)
        cols = slice(h0 * HD, (h0 + 2) * HD)
        wqkv = np.concatenate(
            [Wqkv[:, cols],
             Wqkv[:, DIM + h0 * HD:DIM + (h0 + 2) * HD],
             Wqkv[:, 2 * DIM + h0 * HD:2 * DIM + (h0 + 2) * HD]], axis=1)
        m = {
            "xT": np.ascontiguousarray(x[b].T[:, perm]).astype(BF16),
            "wqkv": np.ascontiguousarray(wqkv).astype(BF16),
            "wout": np.ascontiguousarray(
                Wout[h0 * HD:(h0 + 2) * HD, :]).astype(BF16),
            "maskG": maskG_flat,
        }
        if with_qk_bias:
            m["bqk"] = np.concatenate(
                [bq[h0 * HD:(h0 + 2) * HD],
                 bk[h0 * HD:(h0 + 2) * HD]]).reshape(256, 1).astype(np.float32)
        in_maps.append(m)

    global _last_in_maps
    _last_in_maps = in_maps
    res = run_bass_kernel_spmd(nc, in_maps, core_ids=list(range(NCORES)))

    out = np.zeros((B, S, DIM), np.float32)
    for c in range(NCORES):
        b = c // 4
        part = res.results[c]["outT"].astype(np.float32).T  # (S, DIM) permuted
        out[b][perm] += part
    out += bout[None, None, :]
    if np.any(bv):
        out += (bv @ Wout)[None, None, :]
    return out
